# revision 3
# baseline (speedup 1.0000x reference)
"""BitLinear (absmean ternary quantized linear) on 8 TRN2 NeuronCores.

out[b,t,o] = sum_i x[b,t,i] * (clip(round(W[o,i]/delta), -1, 1) * delta) + bias[o]
delta = mean(|W|) + 1e-8  over the FULL weight.

Sharding: tensor-parallel over OUT rows (11008 / 8 = 1376 rows per core).
x is replicated. delta partial abs-sums are AllReduced across the 8 cores.
Host passes each core its weight shard transposed ([IN, OUT_SH], contiguous)
so the contraction dim lands on SBUF partitions; host concatenates the 8
output shards.

Quantization without round() (not available on any engine):
  q = clip(round(w/d),-1,1) = 1[w >= d/2] - 1[w <= -d/2]      (a.e.)
    = (sign(w - d/2) + sign(w + d/2)) / 2                      (a.e.)
The matmul distributes over the two threshold maps, so each map (exact in
bf16) feeds its own matmul stream:
  psum += xbf @ a.T + xbf @ (-b).T        [DVE is_ge / is_le*-1 method]
  psum += (x/2) @ s1.T + (x/2) @ s2.T     [ACT sign method]
and the epilogue applies out = delta * psum (+ bias via a K=1 PSUM-init
matmul of bias/delta). k-tiles are split between the two methods to
balance ACT vs DVE time. PE is kept HAM-warm through pass A and the
collective gap with cheap chained dummy matmuls.
"""

import numpy as np

B, T, IN, OUT = 8, 16, 4096, 11008
M = B * T               # 128 tokens
CORES = 8
OUT_SH = OUT // CORES   # 1376
KT = IN // 128          # 32 k-tiles
N_TOTAL_W = OUT * IN    # 45088768
EPS = 1e-8

RESIDENT = 26           # k-tiles kept SBUF-resident between pass A and B
NA = 12                 # k-tiles quantized on ACT (sign); rest on DVE (is_ge)
ACT_SET = {round(i * KT / NA) for i in range(NA)}
XH_IDX = {k: i for i, k in enumerate(sorted(ACT_SET))}
COL_SLICES = [(0, 512), (512, 1024), (1024, OUT_SH)]
GAP_CHAIN = 16          # PE<->DVE ping-pong links bridging the collective gap

_CACHE = {}


def _build():
    from concourse import bass, bacc, tile, mybir

    f32 = mybir.dt.float32
    bf16 = mybir.dt.bfloat16
    AF = mybir.ActivationFunctionType
    ALU = mybir.AluOpType

    nc = bacc.Bacc("TRN2", target_bir_lowering=False, debug=False, num_devices=CORES)

    wt_d = nc.dram_tensor("wt", [IN, OUT_SH], f32, kind="ExternalInput")
    xt_d = nc.dram_tensor("xt", [IN, M], f32, kind="ExternalInput")
    bias_d = nc.dram_tensor("bias", [1, OUT_SH], f32, kind="ExternalInput")
    out_d = nc.dram_tensor("out", [M, OUT_SH], f32, kind="ExternalOutput")

    with tile.TileContext(nc) as tc:
        with (
            tc.tile_pool(name="wres", bufs=RESIDENT) as wres,
            tc.tile_pool(name="wstream", bufs=2) as wstream,
            tc.tile_pool(name="xp", bufs=1) as xp,
            tc.tile_pool(name="bp", bufs=1) as bp,
            tc.tile_pool(name="cons", bufs=1) as cons,
            tc.tile_pool(name="stat", bufs=1) as stat,
            tc.tile_pool(name="maps", bufs=2) as maps,
            tc.tile_pool(name="op", bufs=1) as op,
            tc.tile_pool(name="dram", bufs=1, space="DRAM") as dram,
            tc.tile_pool(name="psmall", bufs=1, space="PSUM") as psmall,
            tc.tile_pool(name="pjunk", bufs=1, space="PSUM") as pjunk,
            tc.tile_pool(name="pout", bufs=1, space="PSUM") as pout,
        ):
            # ---- weight DMAs first: they are the memory roofline ----
            w_tiles = {}
            w_all = []
            for k in range(KT):
                if k < RESIDENT:
                    wk = wres.tile([128, OUT_SH], f32, tag="w")
                    w_tiles[k] = wk
                else:
                    wk = wstream.tile([128, OUT_SH], f32, tag="ws")
                nc.sync.dma_start(out=wk[:], in_=wt_d[128 * k : 128 * (k + 1), :])
                w_all.append(wk)

            # ---- constants / small tiles ----
            ones_col = cons.tile([128, 1], f32)
            ones_row = cons.tile([1, 128], f32)
            nc.gpsimd.memset(ones_col[:], 1.0)
            nc.gpsimd.memset(ones_row[:], 1.0)
            warm = cons.tile([128, 1], f32)
            # pre-load the ACT table set containing Sign while DMAs run
            nc.scalar.activation(warm[:], ones_col[:], AF.Sign)

            partials = stat.tile([128, KT], f32)
            sumP = stat.tile([128, 1], f32)
            s_sb = stat.tile([1, 8], f32)
            s_tmp = stat.tile([1, 1], f32)
            gath = stat.tile([1, 8], f32)
            d_sb = stat.tile([1, 1], f32)
            rd_sb = stat.tile([1, 1], f32)
            delta_bc = stat.tile([128, 1], f32)
            th = stat.tile([128, 1], f32)       # +delta/2
            nth = stat.tile([128, 1], f32)      # -delta/2
            junk_sb = stat.tile([128, 1], f32)
            wjunk = stat.tile([1, 8], f32)

            # early dummy collective: wakes ncfw so the real one starts fast
            ccw_in = dram.tile([1, 8], f32)
            ccw_out = dram.tile([1, 8], f32, addr_space="Shared")
            nc.gpsimd.dma_start(out=ccw_in[:], in_=ones_row[0:1, 0:8])
            nc.gpsimd.collective_compute(
                "AllReduce",
                ALU.add,
                replica_groups=[list(range(CORES))],
                ins=[ccw_in[:].opt()],
                outs=[ccw_out[:].opt()],
            )
            nc.gpsimd.dma_start(out=wjunk[:], in_=ccw_out[:])

            # ---- x: load f32, convert to bf16 (and half-scale copy) ----
            bias_sb = bp.tile([1, OUT_SH], f32)
            nc.sync.dma_start(out=bias_sb[:], in_=bias_d[:])
            xstage = xp.tile([128, KT, M], f32)
            nc.sync.dma_start(
                out=xstage[:], in_=xt_d[:].rearrange("(t p) c -> p t c", p=128)
            )
            xbf = xp.tile([128, KT, M], bf16)   # x in bf16 (DVE-method tiles)
            xh = xp.tile([128, NA, M], bf16)    # x/2 in bf16 (ACT-method tiles)
            for k in range(KT):
                if k in ACT_SET:
                    nc.vector.tensor_scalar_mul(
                        xh[:, XH_IDX[k], :], xstage[:, k, :], 0.5
                    )
                else:
                    nc.vector.tensor_copy(xbf[:, k, :], xstage[:, k, :])

            psum_out = pout.tile([M, OUT_SH], f32)
            junk_ps = pjunk.tile([128, 1], f32)

            # ---- pass A: abs-sum each weight tile as it lands ----
            for k in range(KT):
                nc.vector.tensor_reduce(
                    partials[:, k : k + 1],
                    w_all[k][:],
                    axis=mybir.AxisListType.X,
                    op=ALU.add,
                    apply_absolute_value=True,
                )
                # PE warm-keeper: tiny matmul chained on this tile's partial
                nc.tensor.matmul(junk_ps[:], ones_row[:], partials[0:1, k : k + 1])

            # ---- delta: local sum -> AllReduce(x8 replicated) -> bcast ----
            nc.vector.tensor_reduce(
                sumP[:], partials[:], axis=mybir.AxisListType.X, op=ALU.add
            )
            ps1 = psmall.tile([1, 1], f32, tag="ps1")
            nc.tensor.matmul(ps1[:], sumP[:], ones_col[:])  # sum over partitions
            nc.vector.tensor_copy(s_tmp[:], ps1[:])
            nc.vector.tensor_scalar(
                s_sb[:], ones_row[0:1, 0:8], s_tmp[:], None, op0=ALU.mult
            )

            cc_in = dram.tile([1, 8], f32)
            cc_out = dram.tile([1, 8], f32, addr_space="Shared")
            nc.gpsimd.dma_start(out=cc_in[:], in_=s_sb[:])
            nc.gpsimd.collective_compute(
                "AllReduce",
                ALU.add,
                replica_groups=[list(range(CORES))],
                ins=[cc_in[:].opt()],
                outs=[cc_out[:].opt()],
            )
            nc.gpsimd.dma_start(out=gath[:], in_=cc_out[:])

            # PE warm-keeper chain across the collective gap: PE <-> DVE
            # ping-pong; each link's latency spaces the matmuls out in time.
            for _ in range(GAP_CHAIN):
                nc.vector.tensor_copy(junk_sb[:], junk_ps[:])
                nc.tensor.matmul(junk_ps[:], ones_row[:], junk_sb[0:1, 0:1])

            nc.vector.tensor_scalar(
                d_sb[:], gath[0:1, 0:1], 1.0 / N_TOTAL_W, EPS, op0=ALU.mult, op1=ALU.add
            )
            nc.vector.reciprocal(rd_sb[:], d_sb[:])
            psb = psmall.tile([128, 1], f32, tag="psb")
            nc.tensor.matmul(psb[:], ones_row[:], d_sb[:])  # broadcast delta
            nc.vector.tensor_copy(delta_bc[:], psb[:])
            nc.vector.tensor_scalar_mul(th[:], delta_bc[:], 0.5)
            nc.vector.tensor_scalar_mul(nth[:], delta_bc[:], -0.5)

            # bias/delta into PSUM: ones[1,128].T @ biasd[1,N] broadcasts rows
            biasd = bp.tile([1, OUT_SH], f32)
            nc.vector.tensor_scalar(
                biasd[:], bias_sb[:], rd_sb[:], None, op0=ALU.mult
            )
            for c0, c1 in COL_SLICES:
                nc.tensor.matmul(
                    psum_out[:, c0:c1],
                    ones_row[:],
                    biasd[:, c0:c1],
                    start=True,
                    stop=False,
                )

            # ---- pass B: quantize + matmul ----
            for k in range(KT):
                if k in w_tiles:
                    wk = w_tiles[k]
                else:
                    wk = wstream.tile([128, OUT_SH], f32, tag="ws")
                    nc.sync.dma_start(
                        out=wk[:], in_=wt_d[128 * k : 128 * (k + 1), :]
                    )
                mA = maps.tile([128, OUT_SH], bf16, tag="mA")
                mB = maps.tile([128, OUT_SH], bf16, tag="mB")
                if k in ACT_SET:
                    # sign method on ACT; contributes 2q, x carries the 1/2
                    nc.scalar.activation(mA[:], wk[:], AF.Sign, bias=nth[:])
                    nc.scalar.activation(mB[:], wk[:], AF.Sign, bias=th[:])
                    xa = xh[:, XH_IDX[k], :]
                    xb = xa
                else:
                    # threshold method on DVE: q = a - b; minus folded in map
                    nc.vector.tensor_scalar(mA[:], wk[:], th[:], None, op0=ALU.is_ge)
                    nc.vector.tensor_scalar(
                        mB[:], wk[:], nth[:], -1.0, op0=ALU.is_le, op1=ALU.mult
                    )
                    xa = xbf[:, k, :]
                    xb = xa
                last = k == KT - 1
                for c0, c1 in COL_SLICES:
                    nc.tensor.matmul(
                        psum_out[:, c0:c1], xa[:], mA[:, c0:c1], start=False, stop=False
                    )
                for c0, c1 in COL_SLICES:
                    nc.tensor.matmul(
                        psum_out[:, c0:c1], xb[:], mB[:, c0:c1], start=False, stop=last
                    )

            # epilogue: out = delta * psum  (bias already in as bias/delta)
            out_sb = op.tile([M, OUT_SH], f32)
            nc.scalar.activation(
                out_sb[:], psum_out[:], AF.Identity, bias=0.0, scale=delta_bc[:]
            )
            nc.sync.dma_start(out=out_d[:], in_=out_sb[:])

    nc.compile()
    return nc


def _get_nc():
    if "nc" not in _CACHE:
        _CACHE["nc"] = _build()
    return _CACHE["nc"]


def _run(x, weight, bias, **spmd_kwargs):
    from concourse.bass_utils import run_bass_kernel_spmd

    x = np.ascontiguousarray(np.asarray(x), dtype=np.float32)
    weight = np.ascontiguousarray(np.asarray(weight), dtype=np.float32)
    bias = np.ascontiguousarray(np.asarray(bias), dtype=np.float32)

    xt = np.ascontiguousarray(x.reshape(M, IN).T)  # [IN, M]
    in_maps = []
    for c in range(CORES):
        rows = slice(c * OUT_SH, (c + 1) * OUT_SH)
        in_maps.append(
            {
                "xt": xt,
                "wt": np.ascontiguousarray(weight[rows].T),  # [IN, OUT_SH]
                "bias": bias[rows].reshape(1, OUT_SH),
            }
        )
    nc = _get_nc()
    res = run_bass_kernel_spmd(nc, in_maps, core_ids=list(range(CORES)), **spmd_kwargs)
    out = np.concatenate([res.results[c]["out"] for c in range(CORES)], axis=1)
    return out.reshape(B, T, OUT).astype(np.float32), res


def kernel(x, weight, bias):
    out, _ = _run(x, weight, bias)
    return out
